# revision 10
# baseline (speedup 1.0000x reference)
"""YOLOv3 detection-head decode (nn_DetectionLayer) on 8 Trainium2 NeuronCores.

Layout math (per batch image):
  in : prediction [255, 52*52]   (channel-major: ch = a*85 + attr, spatial s = gj*52+gi)
  out: pred       [8112, 85]     (row r = s*3 + a, col = attr)

The decode is fused into PE matmuls that also perform the [ch, s] -> [s, (a,attr)]
transpose. For a 128-wide spatial chunk (M columns of the sigmoided channel tiles):

  psum[s, 0:128]   = sigmoid(chan[0:128])^T   @ R1      (R1 diag: 8,8,0,0,1,...)
  psum[s, 128:255] = sigmoid(chan[128:255])^T @ R2
  psum[s, 85a:85a+4] += box_a^T @ W_a   box_a rows = [8*x_off, 8*y_off, exp(tw), exp(th)]
                                        W_a = diag(1, 1, anc_w[a], anc_h[a])

so PSUM holds final output values in output layout; evict to SBUF and DMA out
contiguously.
"""

import numpy as np

B = 32
C = 255
G = 52
A = 3
ATTRS = 85
S = G * G           # 2704
NCORES = 8
BPC = B // NCORES   # 4 batch images per core
NFULL = S // 128    # 21 full 128-wide spatial chunks
TAIL = S - NFULL * 128  # 16

_CACHE = {}


def _build_bass():
    import concourse.bacc as bacc
    import concourse.tile as tile
    import concourse.bass as bass
    from concourse import mybir

    f32 = mybir.dt.float32
    AF = mybir.ActivationFunctionType

    nc = bacc.Bacc("TRN2", target_bir_lowering=False, debug=False)

    pred = nc.dram_tensor("pred", [BPC, C, S], f32, kind="ExternalInput")
    boxc = nc.dram_tensor("boxc", [2 * A, S], f32, kind="ExternalInput")
    r1d = nc.dram_tensor("r1d", [128, 128], f32, kind="ExternalInput")
    r2d = nc.dram_tensor("r2d", [127, 127], f32, kind="ExternalInput")
    wmd = nc.dram_tensor("wmd", [4, 12], f32, kind="ExternalInput")
    out = nc.dram_tensor("out", [BPC, S * A, ATTRS], f32, kind="ExternalOutput")

    with tile.TileContext(nc) as tc:
        with (
            tc.tile_pool(name="consts", bufs=1) as cpool,
            tc.tile_pool(name="boxp", bufs=1) as bpool,
            tc.tile_pool(name="inp", bufs=2) as ipool,
            tc.tile_pool(name="stg", bufs=2) as spool,
            tc.tile_pool(name="psp", bufs=8, space=bass.MemorySpace.PSUM) as pspool,
        ):
            r1t = cpool.tile([128, 128], f32)
            nc.sync.dma_start(r1t[:], r1d[:])
            r2t = cpool.tile([127, 127], f32)
            nc.sync.dma_start(r2t[:], r2d[:])
            # W blocks at partition base 32a so lhsT/rhs base partitions match
            wmt = cpool.tile([96, 4], f32)
            for a in range(A):
                nc.sync.dma_start(
                    wmt[32 * a : 32 * a + 4, 0:4], wmd[:, 4 * a : 4 * a + 4]
                )

            # gather raw tw/th rows: (b, a) -> pred[b, 85a+2 : 85a+4, :]
            wht = bpool.tile([2 * A * BPC, S], f32)
            for b in range(BPC):
                for a in range(A):
                    g = 2 * (A * b + a)
                    nc.sync.dma_start(
                        wht[g : g + 2, :], pred[b, 85 * a + 2 : 85 * a + 4, :]
                    )
            nc.scalar.activation(wht[:], wht[:], AF.Exp)

            # per-batch box tiles: anchor group a at partitions 32a..32a+3 holds
            # rows [8*x_off, 8*y_off, exp(tw), exp(th)]; offsets from host const,
            # exp rows copied in after ACT exp. Rows outside the groups are
            # never read.
            boxts = []
            for b in range(BPC):
                boxt = bpool.tile([96, S], f32, tag=f"boxt{b}")
                for a in range(A):
                    g = 2 * (A * b + a)
                    nc.sync.dma_start(
                        boxt[32 * a : 32 * a + 2, :], boxc[2 * a : 2 * a + 2, :]
                    )
                    nc.sync.dma_start(
                        boxt[32 * a + 2 : 32 * a + 4, :], wht[g : g + 2, :]
                    )
                boxts.append(boxt)

            for b in range(BPC):
                tA = ipool.tile([128, S], f32, tag="tA")
                tB = ipool.tile([127, S], f32, tag="tB")
                nc.sync.dma_start(tA[:], pred[b, 0:128, :])
                nc.sync.dma_start(tB[:], pred[b, 128:C, :])
                nc.scalar.activation(tA[:], tA[:], AF.Sigmoid)
                nc.scalar.activation(tB[:], tB[:], AF.Sigmoid)

                stg = spool.tile([128, (NFULL + 1) * 255], f32, tag="stg")

                for j in range(NFULL + 1):
                    M = 128 if j < NFULL else TAIL
                    s0 = 128 * j
                    ps = pspool.tile([128, 255], f32, tag="ps")
                    nc.tensor.matmul(
                        ps[:M, 0:128], tA[:, s0 : s0 + M], r1t[:],
                        start=True, stop=False,
                    )
                    nc.tensor.matmul(
                        ps[:M, 128:255], tB[:, s0 : s0 + M], r2t[:],
                        start=False, stop=False,
                    )
                    for a in range(A):
                        nc.tensor.matmul(
                            ps[:M, 85 * a : 85 * a + 4],
                            boxts[b][32 * a : 32 * a + 4, s0 : s0 + M],
                            wmt[32 * a : 32 * a + 4, 0:4],
                            start=False, stop=(a == A - 1),
                        )
                    dst = stg[:M, 255 * j : 255 * (j + 1)]
                    if j % 4 == 3:
                        nc.scalar.copy(dst, ps[:M, 0:255])
                    else:
                        nc.vector.tensor_copy(dst, ps[:M, 0:255])

                # main out DMA: rows 0 .. 3*128*NFULL-1, fully contiguous in dram
                main_dram = out[b, 0 : 3 * 128 * NFULL, :].rearrange(
                    "(j p a) c -> p j a c", j=NFULL, p=128, a=A
                )
                main_sbuf = stg[:, 0 : NFULL * 255].rearrange(
                    "p (j a c) -> p j a c", j=NFULL, a=A, c=ATTRS
                )
                nc.sync.dma_start(main_dram, main_sbuf)
                tail_dram = out[b, 3 * 128 * NFULL : 3 * S, :].rearrange(
                    "(p a) c -> p a c", p=TAIL, a=A
                )
                tail_sbuf = stg[:TAIL, NFULL * 255 : (NFULL + 1) * 255].rearrange(
                    "p (a c) -> p a c", a=A, c=ATTRS
                )
                nc.sync.dma_start(tail_dram, tail_sbuf)

    nc.compile()
    return nc


def get_nc():
    if "nc" not in _CACHE:
        _CACHE["nc"] = _build_bass()
    return _CACHE["nc"]


def make_inputs(prediction, anchors, inp_dim, num_classes):
    """Host-side constant prep + per-core input maps."""
    pred = np.ascontiguousarray(np.asarray(prediction, dtype=np.float32)).reshape(
        B, C, S
    )
    anchors = np.asarray(anchors, dtype=np.float32)
    inp_dim = int(inp_dim)
    num_classes = int(num_classes)
    assert num_classes + 5 == ATTRS
    stride = float(inp_dim // G)

    v = np.ones(ATTRS, np.float32)
    v[0:2] = stride
    v[2:4] = 0.0
    diag = np.concatenate([v, v, v])
    r1 = np.ascontiguousarray(np.diag(diag[0:128]).astype(np.float32))
    r2 = np.ascontiguousarray(np.diag(diag[128:255]).astype(np.float32))

    wm = np.zeros((4, 12), np.float32)
    for a in range(A):
        wm[0, 4 * a + 0] = 1.0
        wm[1, 4 * a + 1] = 1.0
        wm[2, 4 * a + 2] = anchors[a, 0]
        wm[3, 4 * a + 3] = anchors[a, 1]

    s = np.arange(S, dtype=np.float32)
    boxc = np.zeros((2 * A, S), np.float32)
    for a in range(A):
        boxc[2 * a + 0] = (s % G) * stride
        boxc[2 * a + 1] = np.floor(s / G) * stride

    in_maps = [
        {
            "pred": np.ascontiguousarray(pred[BPC * c : BPC * (c + 1)]),
            "boxc": boxc,
            "r1d": r1,
            "r2d": r2,
            "wmd": wm,
        }
        for c in range(NCORES)
    ]
    return in_maps


def kernel(prediction, anchors, inp_dim, num_classes):
    from concourse.bass_utils import run_bass_kernel_spmd

    nc = get_nc()
    in_maps = make_inputs(prediction, anchors, inp_dim, num_classes)
    res = run_bass_kernel_spmd(nc, in_maps, core_ids=list(range(NCORES)))
    out = np.concatenate([r["out"] for r in res.results], axis=0)
    return out.reshape(B, S * A, ATTRS)


# revision 21
# speedup vs baseline: 2.1550x; 2.1550x over previous
"""YOLOv3 detection-head decode (nn_DetectionLayer) on 8 Trainium2 NeuronCores.

Layout math (per batch image):
  in : prediction [255, 52*52]   (channel-major: ch = a*85 + attr, spatial s = gj*52+gi)
  out: pred       [8112, 85]     (row r = s*3 + a, col = attr)

The decode is fused into PE matmuls that also perform the [ch, s] -> [s, (a,attr)]
transpose. Spatial positions are processed in groups of 1024 split 8-way
interleaved (psum partition p of block k holds s = g*1024 + 8p + k) so that each
output-DMA descriptor covers 24 consecutive output rows = 8160 contiguous bytes.

For each (group g, phase k), one 256-wide psum block accumulates 3 matmuls:

  ps[p, 256k + 0:128]   += sig(chan[0:128])[:, s-slice]^T   @ R1   (diag 8,8,0,0,1..)
  ps[p, 256k + 127:255] += sig(chan[127:255])[:, s-slice]^T @ R2   (diag, [0]=0 dup)
  ps[p, 256k + 0:255]   += box[0:12, s-slice]^T @ W12             (0/1 block diag)

box rows (host-computed): [8*x_off, 8*y_off, anc_w*exp(tw), anc_h*exp(th)] per
anchor. PSUM then holds final output values in output layout; evict to SBUF,
DMA out with 8160B-contiguous descriptors.

All DMA slowest-dim (SBUF partition) counts are multiples of 16 so the HWDGE
sprays descriptors across all 16 SDMA rings (127/24-row DMAs serialize on one
ring — measured).
"""

import numpy as np

B = 32
C = 255
G = 52
A = 3
ATTRS = 85
S = G * G            # 2704
NCORES = 8
BPC = B // NCORES    # 4 batch images per core
IK = 8               # spatial interleave factor (consecutive s per out partition)
GRP = 128 * IK       # 1024 spatial positions per full group
NGRP = 3             # groups per batch: 1024 + 1024 + 656
MTAIL = (S - 2 * GRP) // IK  # 82 partitions in the tail group
BLK = 256            # padded psum block width (255 used)

# matmul operand dtype: "f32r" (single-pass, reduced-precision multiply) or
# "f32" (exact, 2-pass LOW_HIGH)
MM_DTYPE = "f32r"

_CACHE = {}


def _build_bass(mm_dtype=None):
    import concourse.bacc as bacc
    import concourse.tile as tile
    import concourse.bass as bass
    from concourse import mybir

    f32 = mybir.dt.float32
    # mdt: dtype for everything feeding the PE. walrus's checkMatmultFP32r
    # requires f32r matmul operands to be *produced* as f32r, so the DMA loads
    # and ACT sigmoid/exp all run on f32r-typed tensors (same 4-byte storage).
    mdt = mybir.dt.float32r if (mm_dtype or MM_DTYPE) == "f32r" else f32
    AF = mybir.ActivationFunctionType

    nc = bacc.Bacc("TRN2", target_bir_lowering=False, debug=False)

    pred = nc.dram_tensor("pred", [BPC, C, S], mdt, kind="ExternalInput")
    boxd = nc.dram_tensor("boxd", [BPC, 8, S], mdt, kind="ExternalInput")
    r1d = nc.dram_tensor("r1d", [128, BLK], mdt, kind="ExternalInput")
    r2d = nc.dram_tensor("r2d", [128, BLK], mdt, kind="ExternalInput")
    w8d = nc.dram_tensor("w8d", [8, BLK], mdt, kind="ExternalInput")
    out = nc.dram_tensor("out", [BPC, S * A, ATTRS], f32, kind="ExternalOutput")

    with tile.TileContext(nc) as tc:
        with (
            tc.tile_pool(name="consts", bufs=1) as cpool,
            tc.tile_pool(name="boxp", bufs=1) as bpool,
            tc.tile_pool(name="inp", bufs=2) as ipool,
            tc.tile_pool(name="stg", bufs=2) as spool,
            tc.tile_pool(name="psp", bufs=2, space=bass.MemorySpace.PSUM) as pspool,
        ):
            r1t = cpool.tile([128, BLK], mdt)
            nc.sync.dma_start(r1t[:], r1d[:])
            r2t = cpool.tile([128, BLK], mdt)
            nc.sync.dma_start(r2t[:], r2d[:])
            w8t = cpool.tile([8, BLK], mdt)
            nc.sync.dma_start(w8t[:], w8d[:])

            # box tile per batch: rows 0..5 = exp of the six tw/th channels,
            # rows 6,7 = [8*x_off, 8*y_off] (shared across anchors via W8).
            # (exp rows first: ACT requires base partition 0/32/64/96.)
            boxts = []
            for b in range(BPC):
                boxt = bpool.tile([16, S], mdt, tag=f"boxt{b}")
                nc.sync.dma_start(boxt[0:8, :], boxd[b])
                nc.scalar.activation(boxt[0:6, :], boxt[0:6, :], AF.Exp)
                boxts.append(boxt)

            for b in range(BPC):
                tA = ipool.tile([128, S], mdt, tag="tA")
                tB = ipool.tile([128, S], mdt, tag="tB")
                nc.sync.dma_start(tA[:], pred[b, 0:128, :])
                nc.sync.dma_start(tB[:], pred[b, 127:255, :])
                nc.scalar.activation(tA[:], tA[:], AF.Sigmoid)
                nc.scalar.activation(tB[:], tB[:], AF.Sigmoid)

                stg = spool.tile([128, NGRP * IK * 255], f32, tag="stg")

                for g in range(NGRP):
                    M = 128 if g < 2 else MTAIL
                    ps = pspool.tile([128, IK * BLK], f32, tag="ps")
                    for k in range(IK):
                        s0 = GRP * g + k
                        sl = slice(s0, s0 + IK * (M - 1) + 1, IK)
                        blk = ps[:M, BLK * k : BLK * (k + 1)]
                        nc.tensor.matmul(
                            blk, tA[:, sl], r1t[:],
                            start=(k % 2 == 0), stop=False,
                        )
                        nc.tensor.matmul(
                            blk, tB[:, sl], r2t[:],
                            start=False, stop=False,
                        )
                        nc.tensor.matmul(
                            blk, boxts[b][0:8, sl], w8t[:],
                            start=False, stop=(k % 2 == 1),
                        )
                    src = ps[:M, :].rearrange("p (k c) -> p k c", k=IK)[:, :, 0:255]
                    dst = stg[:M, 2040 * g : 2040 * (g + 1)].rearrange(
                        "p (k c) -> p k c", c=255
                    )
                    if g % 3 == 2:
                        nc.scalar.copy(dst, src)
                    else:
                        nc.vector.tensor_copy(dst, src)

                # output DMAs: descriptor = 24 consecutive out rows = 8160 B
                main_dram = out[b, 0 : 2 * 3 * GRP, :].rearrange(
                    "(g p i a) c -> p g (i a c)", g=2, p=128, i=IK, a=A
                )
                main_sbuf = stg[:, 0 : 2 * 2040].rearrange(
                    "p (g r) -> p g r", g=2
                )
                nc.sync.dma_start(main_dram, main_sbuf)
                tail0_dram = out[b, 2 * 3 * GRP : 2 * 3 * GRP + 80 * 3 * IK, :].rearrange(
                    "(p i a) c -> p (i a c)", p=80, i=IK, a=A
                )
                nc.sync.dma_start(tail0_dram, stg[0:80, 2 * 2040 : 3 * 2040])
                tail1_dram = out[b, 2 * 3 * GRP + 80 * 3 * IK : 3 * S, :].rearrange(
                    "(p i a) c -> p (i a c)", p=2, i=IK, a=A
                )
                nc.sync.dma_start(
                    tail1_dram, stg[80:MTAIL, 2 * 2040 : 3 * 2040]
                )

    nc.compile()
    return nc


def get_nc(mm_dtype=None):
    key = mm_dtype or MM_DTYPE
    if key not in _CACHE:
        _CACHE[key] = _build_bass(key)
    return _CACHE[key]


def make_inputs(prediction, anchors, inp_dim, num_classes):
    """Host-side constant prep + per-core input maps."""
    pred = np.ascontiguousarray(np.asarray(prediction, dtype=np.float32)).reshape(
        B, C, S
    )
    anchors = np.asarray(anchors, dtype=np.float32)
    inp_dim = int(inp_dim)
    num_classes = int(num_classes)
    assert num_classes + 5 == ATTRS
    stride = float(inp_dim // G)

    # per-channel diag scale: 8,8,0,0,1,1,...
    v = np.ones(C, np.float32)
    for a in range(A):
        v[85 * a + 0 : 85 * a + 2] = stride
        v[85 * a + 2 : 85 * a + 4] = 0.0
    r1 = np.zeros((128, BLK), np.float32)
    r1[np.arange(128), np.arange(128)] = v[0:128]
    r2 = np.zeros((128, BLK), np.float32)
    r2[np.arange(1, 128), np.arange(128, 255)] = v[128:255]  # row j -> col 127+j
    # W8: rows [tw0, th0, tw1, th1, tw2, th2, x8_off, y8_off] -> output cols
    w8 = np.zeros((8, BLK), np.float32)
    for a in range(A):
        w8[2 * a + 0, 85 * a + 2] = anchors[a, 0]
        w8[2 * a + 1, 85 * a + 3] = anchors[a, 1]
        w8[6, 85 * a + 0] = 1.0
        w8[7, 85 * a + 1] = 1.0

    s = np.arange(S, dtype=np.float32)
    boxd = np.zeros((B, 8, S), np.float32)
    for a in range(A):
        boxd[:, 2 * a + 0] = pred[:, 85 * a + 2, :]  # raw tw (exp'd on-chip)
        boxd[:, 2 * a + 1] = pred[:, 85 * a + 3, :]  # raw th
    boxd[:, 6] = (s % G) * stride
    boxd[:, 7] = np.floor(s / G) * stride

    in_maps = [
        {
            "pred": np.ascontiguousarray(pred[BPC * c : BPC * (c + 1)]),
            "boxd": np.ascontiguousarray(boxd[BPC * c : BPC * (c + 1)]),
            "r1d": r1,
            "r2d": r2,
            "w8d": w8,
        }
        for c in range(NCORES)
    ]
    return in_maps


def kernel(prediction, anchors, inp_dim, num_classes):
    from concourse.bass_utils import run_bass_kernel_spmd

    nc = get_nc()
    in_maps = make_inputs(prediction, anchors, inp_dim, num_classes)
    res = run_bass_kernel_spmd(nc, in_maps, core_ids=list(range(NCORES)))
    out = np.concatenate([r["out"] for r in res.results], axis=0)
    return out.reshape(B, S * A, ATTRS)
